# revision 1
# baseline (speedup 1.0000x reference)
"""Trainium2 Bass kernel for nn_LinearReg_55508157333593.

Computes: loss = (c_omega * 0.001 / N) * sum over all rows/groups of
L2 norms of 25-element groups of weight [100000, 800] f32.

Since each row is 32 contiguous groups of 25 floats and rows are contiguous,
the whole buffer is just 3.2M consecutive 25-float groups. We shard the flat
array across 8 NeuronCores (10M floats each) and stream each core's slab
through SBUF as [128, 78125] (each partition owns 3125 consecutive groups).

Raw-Bass manual pipeline (no Tile, no Block barrier), per chunk i:
  SP:  DMA chunk i into input slot i%B         (per-slot completion sems)
  ACT: square chunk i in place (SBUF->SBUF)
  DVE: per-group (25) reduce into this chunk's slice of gs_all [128, 3125]
Endgame: batched ACT sqrts over segments of gs_all (bulk segment overlaps
the stream; the last segment is tiny), each with a fused per-partition
row-sum (accum_out -> pr column), then PE matmul ones.T @ pr -> PSUM,
DVE copy to SBUF, single-partition DMA out. A dummy Sqrt is ACT's first
instruction so one ACT table load (sqrt_and_others, which also contains
square) serves the whole kernel. The host sums the 8 cores' outputs in
float64 and applies the scaling.
"""

import sys

import numpy as np

if "/opt/trn_rl_repo" not in sys.path:
    sys.path.insert(0, "/opt/trn_rl_repo")

N_CORES = 8
P = 128                      # SBUF partitions
GROUP = 25                   # elements per group
C_OMEGA = 0.001
N_ROWS = 100000
ROW = 800                    # elements per row
F_PER_PART = (N_ROWS * ROW) // (N_CORES * P)   # 78125 floats/partition/core

# chunk schedule (floats per partition; multiples of GROUP, sums to 78125):
# big chunks for streaming, finer chunks near the end (quicker input-slot
# turnaround when DVE paces), then a descending tail so the serial compute
# chain after the last DMA byte is short.
SCHEDULE = [3125] * 24 + [625] * 4 + [500, 125]
SEG_BOUNDS = [24, 29, 30]    # sqrt segments: chunks [0,24), [24,29), [29,30)
FIRST_SQRT_AFTER = 26        # emit segment-0 sqrt after this square (overlap)

_compiled = None
LAST_RESULTS = None          # BassKernelResults of the most recent run


def build(f_per_part=F_PER_PART, schedule=None, in_bufs=12, seg_bounds=None,
          first_sqrt_after=None):
    """Build and compile the per-core raw-Bass program."""
    from concourse import bacc, mybir

    if schedule is None:
        schedule = SCHEDULE
        seg_bounds = SEG_BOUNDS
        first_sqrt_after = FIRST_SQRT_AFTER
    n = len(schedule)
    if seg_bounds is None:
        seg_bounds = [max(1, n - 1), n] if n > 1 else [n]
    if first_sqrt_after is None:
        first_sqrt_after = seg_bounds[0]
    assert sum(schedule) == f_per_part
    assert all(s % GROUP == 0 for s in schedule)
    assert seg_bounds[-1] == n and sorted(seg_bounds) == seg_bounds
    assert first_sqrt_after >= seg_bounds[0] - 1
    offs = [sum(schedule[:i]) for i in range(n)]
    gpcs = [s // GROUP for s in schedule]
    goffs = [sum(gpcs[:i]) for i in range(n + 1)]
    total_g = goffs[n]
    n_segs = len(seg_bounds)
    # (end_chunk, gstart, gend) per sqrt segment
    segs = []
    prev = 0
    for b in seg_bounds:
        segs.append((b, goffs[prev], goffs[b]))
        prev = b
    max_sz = max(schedule)
    f32 = mybir.dt.float32
    Act = mybir.ActivationFunctionType

    nc = bacc.Bacc("TRN2", target_bir_lowering=False, debug=False,
                   num_devices=N_CORES)
    x = nc.dram_tensor("x", [P, f_per_part], f32, kind="ExternalInput").ap()
    # single-partition output: one small DMA descriptor, fast completion
    out = nc.dram_tensor("out", [1, n_segs], f32, kind="ExternalOutput").ap()

    B = in_bufs
    # one contiguous ring so a single DVE reduce can span several slots
    ring = nc.alloc_sbuf_tensor("ring", [P, B * max_sz], f32).ap()
    t = [ring[:, b * max_sz:(b + 1) * max_sz] for b in range(B)]

    # one square+reduce PIECE per chunk, except the first two chunks are
    # split in half so DVE's pipeline wakes up earlier (its first wait is
    # released by a half-size square instead of a full one). Grouping
    # several chunks into one reduce was measured slower (backloads DVE).
    pieces = []                  # (chunk, lo, hi) in floats, lo/hi % 25 == 0
    for i in range(n):
        sz = schedule[i]
        if i < 2 and sz >= 2 * GROUP:
            half = (sz // 2 // GROUP) * GROUP
            pieces.append((i, 0, half))
            pieces.append((i, half, sz))
        else:
            pieces.append((i, 0, sz))
    last_piece = {}              # chunk -> index of its last piece
    for p, (c, _, _) in enumerate(pieces):
        last_piece[c] = p
    r_of = last_piece            # reduce ops mirror pieces 1:1

    gs_all = nc.alloc_sbuf_tensor("gs_all", [P, total_g], f32).ap()
    gn = nc.alloc_sbuf_tensor("gn", [P, total_g], f32).ap()
    pr = nc.alloc_sbuf_tensor("pr", [P, n_segs], f32).ap()
    res_sb = nc.alloc_sbuf_tensor("res_sb", [1, n_segs], f32).ap()
    dm = nc.alloc_sbuf_tensor("dm_scratch", [1, 1], f32).ap()
    ps = nc.alloc_psum_tensor("ps", [1, n_segs], f32).ap()
    ones = nc.const_aps.aps[(f32, 1.0)]   # preamble-initialized [128, 1]

    dma_sems = [nc.alloc_semaphore(f"dma_sem{b}") for b in range(B)]
    out_sem = nc.alloc_semaphore("out_sem")
    sq_sem = nc.alloc_semaphore("sq_sem")       # ACT square i done
    red_sem = nc.alloc_semaphore("red_sem")     # DVE reduce i done
    sqrt_sem = nc.alloc_semaphore("sqrt_sem")   # ACT segment sqrts done
    mm_sem = nc.alloc_semaphore("mm_sem")       # PE partition-sum done
    cp_sem = nc.alloc_semaphore("cp_sem")       # PSUM->SBUF copy done

    def emit_sp(sp):
        for i in range(n):
            if i >= B:
                # input slot free once the reduce op covering it completed
                sp.wait_ge(red_sem, r_of[i - B] + 1)
            sp.dma_start(
                t[i % B][:, :schedule[i]], x[:, offs[i]:offs[i] + schedule[i]]
            ).then_inc(dma_sems[i % B], 16)
        sp.wait_ge(cp_sem, 1)
        sp.dma_start(out, res_sb).then_inc(out_sem, 16)
        sp.wait_ge(out_sem, 16)

    def emit_act(act):
        # table prefetch: first activation is a Sqrt, so the one table set
        # loaded (sqrt_and_others) also covers Square -> no mid-kernel load
        act.activation(dm, ones[0:1, :], Act.Sqrt)

        def emit_seg(s):
            end_chunk, glo, ghi = segs[s]
            act.wait_ge(red_sem, r_of[end_chunk - 1] + 1)
            act.activation(gn[:, glo:ghi], gs_all[:, glo:ghi], Act.Sqrt,
                           accum_out=pr[:, s:s + 1]).then_inc(sqrt_sem, 1)

        emitted = 0
        prev_chunk = -1
        for c, lo, hi in pieces:
            if c != prev_chunk:
                if (emitted == 0 and prev_chunk >= first_sqrt_after
                        and n_segs > 1):
                    emit_seg(0)
                    emitted = 1
                act.wait_ge(dma_sems[c % B], 16 * (c // B + 1))
                prev_chunk = c
            act.activation(t[c % B][:, lo:hi], t[c % B][:, lo:hi],
                           Act.Square).then_inc(sq_sem, 1)
        for s in range(emitted, n_segs):
            emit_seg(s)

    def emit_dve(dve):
        for p, (c, lo, hi) in enumerate(pieces):
            dve.wait_ge(sq_sem, p + 1)
            base = (c % B) * max_sz
            dve.reduce_sum(
                gs_all[:, goffs[c] + lo // GROUP:goffs[c] + hi // GROUP],
                ring[:, base + lo:base + hi].rearrange("p (g k) -> p g k",
                                                       k=GROUP),
                axis=mybir.AxisListType.X,
            ).then_inc(red_sem, 1)
        dve.wait_ge(mm_sem, 1)
        dve.tensor_copy(res_sb, ps).then_inc(cp_sem, 1)

    def emit_pe(pe):
        pe.wait_ge(sqrt_sem, n_segs)
        pe.matmul(ps, ones, pr, start=True, stop=True).then_inc(mm_sem, 1)

    emit_sp(nc.sync)
    emit_act(nc.scalar)
    emit_dve(nc.vector)
    emit_pe(nc.tensor)

    nc.compile()
    return nc


def kernel(weight, c_omega):
    global _compiled, LAST_RESULTS
    from concourse.bass_utils import run_bass_kernel_spmd

    if _compiled is None:
        _compiled = build()
    nc = _compiled

    w = np.asarray(weight)
    if w.dtype != np.float32:
        w = w.astype(np.float32)
    w = np.ascontiguousarray(w)
    flat = w.reshape(-1)
    per_core = flat.size // N_CORES
    in_maps = [
        {"x": flat[c * per_core:(c + 1) * per_core].reshape(P, F_PER_PART)}
        for c in range(N_CORES)
    ]
    LAST_RESULTS = run_bass_kernel_spmd(nc, in_maps,
                                        core_ids=list(range(N_CORES)))
    total = 0.0
    for r in LAST_RESULTS.results:
        total += float(r["out"].astype(np.float64).sum())
    loss = total / N_ROWS * (C_OMEGA * float(c_omega))
    return np.float32(loss)


def selftest_sim(f_per_part=625, schedule=(250, 250, 75, 25, 25),
                 in_bufs=3, seed=0, **kw):
    """CoreSim check on a scaled-down instance; returns max rel err."""
    from concourse.bass_interp import CoreSim

    nc = build(f_per_part=f_per_part, schedule=list(schedule),
               in_bufs=in_bufs, **kw)
    rng = np.random.default_rng(seed)
    xv = rng.standard_normal((P, f_per_part)).astype(np.float32)
    sim = CoreSim(nc)
    sim.tensor("x")[:] = xv
    sim.simulate()
    got = float(np.array(sim.tensor("out")).astype(np.float64).sum())
    g = xv.reshape(P, f_per_part // GROUP, GROUP)
    want = float(np.sqrt((g.astype(np.float64) ** 2).sum(-1)).sum())
    return abs(got - want) / abs(want)



# revision 7
# speedup vs baseline: 1.1339x; 1.1339x over previous
"""Trainium2 Bass kernel for nn_LinearReg_55508157333593.

Computes: loss = (c_omega * 0.001 / N) * sum over all rows/groups of
L2 norms of 25-element groups of weight [100000, 800] f32.

Since each row is 32 contiguous groups of 25 floats and rows are contiguous,
the whole buffer is just 3.2M consecutive 25-float groups. We shard the flat
array across 8 NeuronCores (10M floats each) and stream each core's slab
through SBUF as [128, 78125] (each partition owns 3125 consecutive groups).

Raw-Bass manual pipeline (no Tile, no Block barrier), per chunk i:
  SP:  DMA chunk i into input slot i%B       (per-slot completion sems)
  ACT: square chunk i f32 -> bf16 sq ring    (halves DVE input width)
  DVE: per-group (25) reduce bf16 -> f32 gs_all [128, 3125]
Squaring to bf16 doubles DVE reduce throughput, so the reduce pipeline
tracks the DMA stream instead of lagging it (the f32 version's reduces
ran ~4.1us/chunk vs 4.4us of DMA and finished ~12us after the stream).
The schedule ends with a run of small chunks so the last big reduce
completes while small chunks still stream.

Endgame: ACT sqrts gs_all segments in place (bulk segments overlap the
stream), each with a fused per-partition row-sum (accum_out -> pr column),
then ACT itself DMAs pr [128, n_segs] out. The host sums partitions/
segments/cores in float64 and applies the scaling (the "all-reduce the
scalar" gather step). A dummy Sqrt is ACT's first instruction so one ACT
table load (sqrt_and_others, which also contains Square) serves the whole
kernel.

bf16 precision note: squares are quantized to 8 mantissa bits (~0.4%
relative) but group sums accumulate in f32 and the loss is a mean over
3.2M groups, so quantization noise averages out (measured ~1e-4 rel err,
tolerance 2e-2).
"""

import sys
import types

import numpy as np

if "/opt/trn_rl_repo" not in sys.path:
    sys.path.insert(0, "/opt/trn_rl_repo")


def _ensure_axon_ntff_hook():
    """Provide antenv.axon_hooks if the image's antenv lacks it.

    concourse.bass_utils imports it when BASS_TRACE is set under axon;
    without it the run crashes. If we install the shim we also register
    the ctypes-based NTFF hook the way trn_agent_boot.boot would have.
    """
    try:
        import antenv.axon_hooks  # noqa: F401
        return
    except ImportError:
        pass
    try:
        import antenv
    except ImportError:
        return
    mod = types.ModuleType("antenv.axon_hooks")
    mod._hook = None

    def set_axon_ntff_profile_hook(hook):
        mod._hook = hook

    def get_axon_ntff_profile_hook():
        return mod._hook

    mod.set_axon_ntff_profile_hook = set_axon_ntff_profile_hook
    mod.get_axon_ntff_profile_hook = get_axon_ntff_profile_hook
    sys.modules["antenv.axon_hooks"] = mod
    antenv.axon_hooks = mod
    try:
        from trn_agent_boot.trn_boot import _ntff_profile_via_ctypes

        hook = _ntff_profile_via_ctypes("/opt/axon/libaxon_pjrt.so")
        if hook is not None:
            set_axon_ntff_profile_hook(hook)
    except Exception:
        pass


_ensure_axon_ntff_hook()

N_CORES = 8
P = 128                      # SBUF partitions
GROUP = 25                   # elements per group
C_OMEGA = 0.001
N_ROWS = 100000
ROW = 800                    # elements per row
F_PER_PART = (N_ROWS * ROW) // (N_CORES * P)   # 78125 floats/partition/core

# chunk schedule (floats per partition; multiples of GROUP, sums to 78125):
# big chunks for streaming, then a long small-chunk tail so the last big
# reduce (the slowest compute op) finishes while small chunks still stream
# and the serial chain after the last DMA byte stays short.
SCHEDULE = [3125] * 23 + [625] * 8 + [500, 250, 250, 125, 125]
# sqrt segments in chunk indices: [0,6), [6,12), [12,18), [18,23), [23,36)
SEG_BOUNDS = [6, 12, 18, 23, 36]
# seg index -> emit its sqrt just before this chunk's square piece(s)
SEG_EMIT_BEFORE = {0: 9, 1: 15, 2: 21, 3: 32}

_compiled = None
LAST_RESULTS = None          # BassKernelResults of the most recent run


def build(f_per_part=F_PER_PART, schedule=None, in_bufs=8, sq_bufs=8,
          seg_bounds=None, seg_emit_before=None):
    """Build and compile the per-core raw-Bass program."""
    from concourse import bacc, mybir

    if schedule is None:
        schedule = SCHEDULE
        seg_bounds = SEG_BOUNDS
        seg_emit_before = SEG_EMIT_BEFORE
    n = len(schedule)
    if seg_bounds is None:
        seg_bounds = [n]
    if seg_emit_before is None:
        seg_emit_before = {}
    assert sum(schedule) == f_per_part
    assert all(s % GROUP == 0 for s in schedule)
    assert seg_bounds[-1] == n and sorted(seg_bounds) == seg_bounds
    offs = [sum(schedule[:i]) for i in range(n)]
    gpcs = [s // GROUP for s in schedule]
    goffs = [sum(gpcs[:i]) for i in range(n + 1)]
    total_g = goffs[n]
    n_segs = len(seg_bounds)
    # (end_chunk, gstart, gend) per sqrt segment
    segs = []
    prev = 0
    for b in seg_bounds:
        segs.append((b, goffs[prev], goffs[b]))
        prev = b
    max_sz = max(schedule)
    f32 = mybir.dt.float32
    bf16 = mybir.dt.bfloat16
    Act = mybir.ActivationFunctionType

    nc = bacc.Bacc("TRN2", target_bir_lowering=False, debug=False,
                   num_devices=N_CORES)
    x = nc.dram_tensor("x", [P, f_per_part], f32, kind="ExternalInput").ap()
    out = nc.dram_tensor("out", [P, n_segs], f32, kind="ExternalOutput").ap()

    Bi = in_bufs
    Bs = sq_bufs
    in_ring = nc.alloc_sbuf_tensor("in_ring", [P, Bi * max_sz], f32).ap()
    sq_ring = nc.alloc_sbuf_tensor("sq_ring", [P, Bs * max_sz], bf16).ap()
    t_in = [in_ring[:, b * max_sz:(b + 1) * max_sz] for b in range(Bi)]
    t_sq = [sq_ring[:, b * max_sz:(b + 1) * max_sz] for b in range(Bs)]

    # one square+reduce PIECE per chunk, except the first two chunks are
    # split in half so DVE's pipeline wakes up earlier (its first wait is
    # released by a half-size square instead of a full one).
    pieces = []                  # (chunk, lo, hi) in floats, lo/hi % 25 == 0
    for i in range(n):
        sz = schedule[i]
        if i < 2 and sz >= 2 * GROUP:
            half = (sz // 2 // GROUP) * GROUP
            pieces.append((i, 0, half))
            pieces.append((i, half, sz))
        else:
            pieces.append((i, 0, sz))
    last_piece = {}              # chunk -> index of its last piece
    for pidx, (c, _, _) in enumerate(pieces):
        last_piece[c] = pidx
    r_of = last_piece            # reduce ops mirror pieces 1:1

    gs_all = nc.alloc_sbuf_tensor("gs_all", [P, total_g], f32).ap()
    pr = nc.alloc_sbuf_tensor("pr", [P, n_segs], f32).ap()
    dm = nc.alloc_sbuf_tensor("dm_scratch", [1, 1], f32).ap()
    ones = nc.const_aps.aps[(f32, 1.0)]   # preamble-initialized [128, 1]

    dma_sems = [nc.alloc_semaphore(f"dma_sem{b}") for b in range(Bi)]
    out_sem = nc.alloc_semaphore("out_sem")
    sq_sem = nc.alloc_semaphore("sq_sem")       # ACT square piece done
    red_sem = nc.alloc_semaphore("red_sem")     # DVE reduce piece done
    sqrt_sem = nc.alloc_semaphore("sqrt_sem")   # ACT segment sqrts done

    def emit_sp(sp):
        for i in range(n):
            if i >= Bi:
                # input slot free once ACT consumed the chunk B_in back
                sp.wait_ge(sq_sem, r_of[i - Bi] + 1)
            sp.dma_start(
                t_in[i % Bi][:, :schedule[i]],
                x[:, offs[i]:offs[i] + schedule[i]],
            ).then_inc(dma_sems[i % Bi], 16)

    def emit_act(act):
        # table prefetch: first activation is a Sqrt, so the one table set
        # loaded (sqrt_and_others) also covers Square -> no mid-kernel load
        act.activation(dm, ones[0:1, :], Act.Sqrt)

        emitted = 0

        def emit_seg(s):
            end_chunk, glo, ghi = segs[s]
            act.wait_ge(red_sem, r_of[end_chunk - 1] + 1)
            act.activation(gs_all[:, glo:ghi], gs_all[:, glo:ghi], Act.Sqrt,
                           accum_out=pr[:, s:s + 1]).then_inc(sqrt_sem, 1)

        prev_chunk = -1
        for c, lo, hi in pieces:
            if c != prev_chunk:
                while emitted < n_segs and seg_emit_before.get(emitted) == c:
                    emit_seg(emitted)
                    emitted += 1
                act.wait_ge(dma_sems[c % Bi], 16 * (c // Bi + 1))
                if c >= Bs:
                    # sq slot free once DVE reduced the chunk B_sq back
                    act.wait_ge(red_sem, r_of[c - Bs] + 1)
                prev_chunk = c
            act.activation(t_sq[c % Bs][:, lo:hi], t_in[c % Bi][:, lo:hi],
                           Act.Square).then_inc(sq_sem, 1)
        for s in range(emitted, n_segs):
            emit_seg(s)
        # pr's accumulator writes must fully retire before the DMA engines
        # read it (dma_start only enqueues; same-engine order isn't enough)
        act.wait_ge(sqrt_sem, n_segs)
        act.dma_start(out, pr).then_inc(out_sem, 16)
        act.wait_ge(out_sem, 16)

    def emit_dve(dve):
        for pidx, (c, lo, hi) in enumerate(pieces):
            dve.wait_ge(sq_sem, pidx + 1)
            base = (c % Bs) * max_sz
            dve.reduce_sum(
                gs_all[:, goffs[c] + lo // GROUP:goffs[c] + hi // GROUP],
                sq_ring[:, base + lo:base + hi].rearrange(
                    "p (g k) -> p g k", k=GROUP),
                axis=mybir.AxisListType.X,
            ).then_inc(red_sem, 1)

    emit_sp(nc.sync)
    emit_act(nc.scalar)
    emit_dve(nc.vector)

    nc.compile()
    return nc


def kernel(weight, c_omega):
    global _compiled, LAST_RESULTS
    from concourse.bass_utils import run_bass_kernel_spmd

    if _compiled is None:
        _compiled = build()
    nc = _compiled

    w = np.asarray(weight)
    if w.dtype != np.float32:
        w = w.astype(np.float32)
    w = np.ascontiguousarray(w)
    flat = w.reshape(-1)
    per_core = flat.size // N_CORES
    in_maps = [
        {"x": flat[c * per_core:(c + 1) * per_core].reshape(P, F_PER_PART)}
        for c in range(N_CORES)
    ]
    LAST_RESULTS = run_bass_kernel_spmd(nc, in_maps,
                                        core_ids=list(range(N_CORES)))
    total = 0.0
    for r in LAST_RESULTS.results:
        total += float(r["out"].astype(np.float64).sum())
    loss = total / N_ROWS * (C_OMEGA * float(c_omega))
    return np.float32(loss)


def selftest_sim(f_per_part=625, schedule=(250, 250, 75, 25, 25),
                 in_bufs=3, sq_bufs=3, seed=0, **kw):
    """CoreSim check on a scaled-down instance; returns max rel err."""
    from concourse.bass_interp import CoreSim

    nc = build(f_per_part=f_per_part, schedule=list(schedule),
               in_bufs=in_bufs, sq_bufs=sq_bufs, **kw)
    rng = np.random.default_rng(seed)
    xv = rng.standard_normal((P, f_per_part)).astype(np.float32)
    sim = CoreSim(nc)
    sim.tensor("x")[:] = xv
    sim.simulate()
    got = float(np.array(sim.tensor("out")).astype(np.float64).sum())
    g = xv.reshape(P, f_per_part // GROUP, GROUP)
    want = float(np.sqrt((g.astype(np.float64) ** 2).sum(-1)).sum())
    return abs(got - want) / abs(want)


# revision 12
# speedup vs baseline: 1.2358x; 1.0898x over previous
"""Trainium2 Bass kernel for nn_LinearReg_55508157333593.

Computes: loss = (c_omega * 0.001 / N) * sum over all rows/groups of
L2 norms of 25-element groups of weight [100000, 800] f32.

Since each row is 32 contiguous groups of 25 floats and rows are contiguous,
the whole buffer is just 3.2M consecutive 25-float groups. We shard the flat
array across 8 NeuronCores (10M floats each) and stream each core's slab
through SBUF as [128, 78125] (each partition owns 3125 consecutive groups).

Raw-Bass manual pipeline (no Tile, no Block barrier), per chunk i:
  SP:  DMA chunk i into input slot i%B       (per-slot completion sems)
  ACT: square chunk i f32 -> bf16 sq ring    (halves DVE input width)
  DVE: per-group (25) reduce bf16 -> f32 gs_all [128, 3125]
Squaring to bf16 doubles DVE reduce throughput, so the reduce pipeline
tracks the DMA stream instead of lagging it (the f32 version's reduces
ran ~4.1us/chunk vs 4.4us of DMA and finished ~12us after the stream).
The schedule ends with a run of small chunks so the last big reduce
completes while small chunks still stream.

Endgame: ACT sqrts gs_all segments in place (bulk segments overlap the
stream), each with a fused per-partition row-sum (accum_out -> pr column),
then ACT itself DMAs pr [128, n_segs] out. The host sums partitions/
segments/cores in float64 and applies the scaling (the "all-reduce the
scalar" gather step). A dummy Sqrt is ACT's first instruction so one ACT
table load (sqrt_and_others, which also contains Square) serves the whole
kernel.

bf16 precision note: squares are quantized to 8 mantissa bits (~0.4%
relative) but group sums accumulate in f32 and the loss is a mean over
3.2M groups, so quantization noise averages out (measured ~1e-4 rel err,
tolerance 2e-2).
"""

import sys
import types

import numpy as np

if "/opt/trn_rl_repo" not in sys.path:
    sys.path.insert(0, "/opt/trn_rl_repo")


def _ensure_axon_ntff_hook():
    """Provide antenv.axon_hooks if the image's antenv lacks it.

    concourse.bass_utils imports it when BASS_TRACE is set under axon;
    without it the run crashes. If we install the shim we also register
    the ctypes-based NTFF hook the way trn_agent_boot.boot would have.
    """
    try:
        import antenv.axon_hooks  # noqa: F401
        return
    except ImportError:
        pass
    try:
        import antenv
    except ImportError:
        return
    mod = types.ModuleType("antenv.axon_hooks")
    mod._hook = None

    def set_axon_ntff_profile_hook(hook):
        mod._hook = hook

    def get_axon_ntff_profile_hook():
        return mod._hook

    mod.set_axon_ntff_profile_hook = set_axon_ntff_profile_hook
    mod.get_axon_ntff_profile_hook = get_axon_ntff_profile_hook
    sys.modules["antenv.axon_hooks"] = mod
    antenv.axon_hooks = mod
    try:
        from trn_agent_boot.trn_boot import _ntff_profile_via_ctypes

        hook = _ntff_profile_via_ctypes("/opt/axon/libaxon_pjrt.so")
        if hook is not None:
            set_axon_ntff_profile_hook(hook)
    except Exception:
        pass


_ensure_axon_ntff_hook()

N_CORES = 8
P = 128                      # SBUF partitions
GROUP = 25                   # elements per group
C_OMEGA = 0.001
N_ROWS = 100000
ROW = 800                    # elements per row
F_PER_PART = (N_ROWS * ROW) // (N_CORES * P)   # 78125 floats/partition/core

# chunk schedule (floats per partition; multiples of GROUP, sums to 78125):
# big chunks for streaming, then a long small-chunk tail so the last big
# reduce (the slowest compute op) finishes while small chunks still stream
# and the serial chain after the last DMA byte stays short.
SCHEDULE = [3125] * 23 + [625] * 8 + [500, 250, 250, 125, 125]
# sqrt segments in chunk indices
SEG_BOUNDS = [6, 12, 18, 24, 34, 36]
# seg index -> emit its sqrt just before this chunk's square piece(s)
SEG_EMIT_BEFORE = {0: 9, 1: 15, 2: 21, 3: 31, 4: 35}
# chunks whose input DMA is issued from ACT's HWDGE queue instead of SP's:
# two descriptor generators fill the SDMA rings in parallel at the start,
# so all 16 engines come online ~2x sooner (the ramp was ~3us single-queue)
ACT_DMA_CHUNKS = (1, 3)

_compiled = None
LAST_RESULTS = None          # BassKernelResults of the most recent run


def build(f_per_part=F_PER_PART, schedule=None, in_bufs=8, sq_bufs=8,
          seg_bounds=None, seg_emit_before=None, act_dma_chunks=None):
    """Build and compile the per-core raw-Bass program."""
    from concourse import bacc, mybir

    if schedule is None:
        schedule = SCHEDULE
        seg_bounds = SEG_BOUNDS
        seg_emit_before = SEG_EMIT_BEFORE
        act_dma_chunks = ACT_DMA_CHUNKS
    if act_dma_chunks is None:
        act_dma_chunks = ()
    act_dma_chunks = set(act_dma_chunks)
    assert all(c < in_bufs for c in act_dma_chunks), \
        "ACT-issued chunks must be first-use slots (no reuse wait on ACT)"
    n = len(schedule)
    if seg_bounds is None:
        seg_bounds = [n]
    if seg_emit_before is None:
        seg_emit_before = {}
    assert sum(schedule) == f_per_part
    assert all(s % GROUP == 0 for s in schedule)
    assert seg_bounds[-1] == n and sorted(seg_bounds) == seg_bounds
    offs = [sum(schedule[:i]) for i in range(n)]
    gpcs = [s // GROUP for s in schedule]
    goffs = [sum(gpcs[:i]) for i in range(n + 1)]
    total_g = goffs[n]
    n_segs = len(seg_bounds)
    # (end_chunk, gstart, gend) per sqrt segment
    segs = []
    prev = 0
    for b in seg_bounds:
        segs.append((b, goffs[prev], goffs[b]))
        prev = b
    max_sz = max(schedule)
    f32 = mybir.dt.float32
    bf16 = mybir.dt.bfloat16
    Act = mybir.ActivationFunctionType

    nc = bacc.Bacc("TRN2", target_bir_lowering=False, debug=False,
                   num_devices=N_CORES)
    x = nc.dram_tensor("x", [P, f_per_part], f32, kind="ExternalInput").ap()
    out = nc.dram_tensor("out", [P, n_segs], f32, kind="ExternalOutput").ap()

    Bi = in_bufs
    Bs = sq_bufs
    in_ring = nc.alloc_sbuf_tensor("in_ring", [P, Bi * max_sz], f32).ap()
    sq_ring = nc.alloc_sbuf_tensor("sq_ring", [P, Bs * max_sz], bf16).ap()
    t_in = [in_ring[:, b * max_sz:(b + 1) * max_sz] for b in range(Bi)]
    t_sq = [sq_ring[:, b * max_sz:(b + 1) * max_sz] for b in range(Bs)]

    # one square+reduce PIECE per chunk, except the first two chunks are
    # split in half so DVE's pipeline wakes up earlier (its first wait is
    # released by a half-size square instead of a full one).
    pieces = []                  # (chunk, lo, hi) in floats, lo/hi % 25 == 0
    for i in range(n):
        sz = schedule[i]
        if i < 2 and sz >= 2 * GROUP:
            half = (sz // 2 // GROUP) * GROUP
            pieces.append((i, 0, half))
            pieces.append((i, half, sz))
        else:
            pieces.append((i, 0, sz))
    last_piece = {}              # chunk -> index of its last piece
    for pidx, (c, _, _) in enumerate(pieces):
        last_piece[c] = pidx
    r_of = last_piece            # reduce ops mirror pieces 1:1

    gs_all = nc.alloc_sbuf_tensor("gs_all", [P, total_g], f32).ap()
    pr = nc.alloc_sbuf_tensor("pr", [P, n_segs], f32).ap()
    dm = nc.alloc_sbuf_tensor("dm_scratch", [1, 1], f32).ap()
    ones = nc.const_aps.aps[(f32, 1.0)]   # preamble-initialized [128, 1]

    dma_sems = [nc.alloc_semaphore(f"dma_sem{b}") for b in range(Bi)]
    out_sem = nc.alloc_semaphore("out_sem")
    sq_sem = nc.alloc_semaphore("sq_sem")       # ACT square piece done
    red_sem = nc.alloc_semaphore("red_sem")     # DVE reduce piece done
    sqrt_sem = nc.alloc_semaphore("sqrt_sem")   # ACT segment sqrts done

    def emit_sp(sp):
        for i in range(n):
            if i in act_dma_chunks:
                continue
            if i >= Bi:
                # input slot free once ACT consumed the chunk B_in back
                sp.wait_ge(sq_sem, r_of[i - Bi] + 1)
            sp.dma_start(
                t_in[i % Bi][:, :schedule[i]],
                x[:, offs[i]:offs[i] + schedule[i]],
            ).then_inc(dma_sems[i % Bi], 16)
        # final output: pr's accumulator writes retire (sqrt_sem counts the
        # per-segment ACTIVATION_READ_ACCUMULATORs) -> DMA pr out
        sp.wait_ge(sqrt_sem, n_segs)
        sp.dma_start(out, pr).then_inc(out_sem, 16)
        sp.wait_ge(out_sem, 16)

    def emit_act(act):
        # early-chunk DMAs on ACT's own HWDGE queue (parallel descriptor
        # generation with SP's queue during the ramp)
        for c in sorted(act_dma_chunks):
            act.dma_start(
                t_in[c % Bi][:, :schedule[c]],
                x[:, offs[c]:offs[c] + schedule[c]],
            ).then_inc(dma_sems[c % Bi], 16)
        # table prefetch: first activation is a Sqrt, so the one table set
        # loaded (sqrt_and_others) also covers Square -> no mid-kernel load
        act.activation(dm, ones[0:1, :], Act.Sqrt)

        emitted = 0

        def emit_seg(s):
            end_chunk, glo, ghi = segs[s]
            act.wait_ge(red_sem, r_of[end_chunk - 1] + 1)
            act.activation(gs_all[:, glo:ghi], gs_all[:, glo:ghi], Act.Sqrt,
                           accum_out=pr[:, s:s + 1]).then_inc(sqrt_sem, 1)

        prev_chunk = -1
        for c, lo, hi in pieces:
            if c != prev_chunk:
                while emitted < n_segs and seg_emit_before.get(emitted) == c:
                    emit_seg(emitted)
                    emitted += 1
                act.wait_ge(dma_sems[c % Bi], 16 * (c // Bi + 1))
                if c >= Bs:
                    # sq slot free once DVE reduced the chunk B_sq back
                    act.wait_ge(red_sem, r_of[c - Bs] + 1)
                prev_chunk = c
            act.activation(t_sq[c % Bs][:, lo:hi], t_in[c % Bi][:, lo:hi],
                           Act.Square).then_inc(sq_sem, 1)
        for s in range(emitted, n_segs):
            emit_seg(s)

    def emit_dve(dve):
        for pidx, (c, lo, hi) in enumerate(pieces):
            dve.wait_ge(sq_sem, pidx + 1)
            base = (c % Bs) * max_sz
            dve.reduce_sum(
                gs_all[:, goffs[c] + lo // GROUP:goffs[c] + hi // GROUP],
                sq_ring[:, base + lo:base + hi].rearrange(
                    "p (g k) -> p g k", k=GROUP),
                axis=mybir.AxisListType.X,
            ).then_inc(red_sem, 1)

    emit_sp(nc.sync)
    emit_act(nc.scalar)
    emit_dve(nc.vector)

    nc.compile()
    return nc


def kernel(weight, c_omega):
    global _compiled, LAST_RESULTS
    from concourse.bass_utils import run_bass_kernel_spmd

    if _compiled is None:
        _compiled = build()
    nc = _compiled

    w = np.asarray(weight)
    if w.dtype != np.float32:
        w = w.astype(np.float32)
    w = np.ascontiguousarray(w)
    flat = w.reshape(-1)
    per_core = flat.size // N_CORES
    in_maps = [
        {"x": flat[c * per_core:(c + 1) * per_core].reshape(P, F_PER_PART)}
        for c in range(N_CORES)
    ]
    LAST_RESULTS = run_bass_kernel_spmd(nc, in_maps,
                                        core_ids=list(range(N_CORES)))
    total = 0.0
    for r in LAST_RESULTS.results:
        total += float(r["out"].astype(np.float64).sum())
    loss = total / N_ROWS * (C_OMEGA * float(c_omega))
    return np.float32(loss)


def selftest_sim(f_per_part=625, schedule=(250, 250, 75, 25, 25),
                 in_bufs=3, sq_bufs=3, seed=0, **kw):
    """CoreSim check on a scaled-down instance; returns max rel err."""
    from concourse.bass_interp import CoreSim

    nc = build(f_per_part=f_per_part, schedule=list(schedule),
               in_bufs=in_bufs, sq_bufs=sq_bufs, **kw)
    rng = np.random.default_rng(seed)
    xv = rng.standard_normal((P, f_per_part)).astype(np.float32)
    sim = CoreSim(nc)
    sim.tensor("x")[:] = xv
    sim.simulate()
    got = float(np.array(sim.tensor("out")).astype(np.float64).sum())
    g = xv.reshape(P, f_per_part // GROUP, GROUP)
    want = float(np.sqrt((g.astype(np.float64) ** 2).sum(-1)).sum())
    return abs(got - want) / abs(want)
